# revision 1
# baseline (speedup 1.0000x reference)
"""Trainium2 Bass kernel: causal multi-head self-attention (B=1, S=4096, E=768, H=12).

Sharding over 8 NeuronCores: 4 head-groups (3 heads each; tensor-parallel over
QKV output columns / WO rows) x 2 query shards (interleaved 128-row blocks of
the sequence, for causal load balance).  Host pre-transposes activations to
feature-major layout, slices weights per core, and sums the 4 head-group
partial outputs at the end.

Device kernel (per core, identical SPMD program):
  - QK projections emit transposed outputs Q^T/K^T [d, s] directly
    (lhsT = weight columns, rhs = x^T) -- heads 0/1 packed on partition halves.
  - V projection emits natural [s, d] (lhsT = v^T chunk, rhs = wv), with a
    ones-column appended per head so the PV matmul also produces the softmax
    denominator (M = 65).
  - Logits are computed transposed ([k, q]); causal masking uses 8 per-core
    bias tiles indexed by (kappa - 8*tau); exp runs on ACT batched 2 key-blocks
    wide; unnormalized O'^T accumulates in PSUM over kappa.
  - 1/sigma is broadcast across partitions with a K=1 PE matmul and applied
    during the PSUM->SBUF copy; output projection accumulates the 3 heads into
    [q, e]-layout PSUM and is DMA'd out.

All matmul operands are bitcast to float32r (full-rate PE, fp32 accumulate).
"""

import os
import sys

import numpy as np

for _p in ("/opt/trn_rl_repo", "/root/.axon_site/_ro/trn_rl_repo"):
    if os.path.isdir(_p) and _p not in sys.path:
        sys.path.insert(0, _p)

import concourse.bass as bass  # noqa: E402
import concourse.mybir as mybir  # noqa: E402
import concourse.tile as tile  # noqa: E402
from concourse import bacc  # noqa: E402

F32 = mybir.dt.float32
F32R = mybir.dt.float32r
AF = mybir.ActivationFunctionType

EMBED = 768
NHEADS = 12
DH = 64
HG = 4  # head groups (tensor parallel)
QS = 2  # query shards (interleaved 128-blocks)
GH = NHEADS // HG  # heads per group (3)
GW = GH * DH  # group width (192)
NE = EMBED // 128  # contraction chunks (6)
NEG = -1.0e9
SCALE = 1.0 / 8.0  # 1/sqrt(DH)
S_FULL = 4096


def build_nc(S):
    """Build the per-core SPMD Bass program for sequence length S."""
    NB = S // 128  # key blocks
    NQB = NB // QS  # query blocks per core
    NT = NQB // 4  # local q-tiles of 512
    SQ = NQB * 128

    nc = bacc.Bacc("TRN2", target_bir_lowering=False, debug=False,
                   num_devices=HG * QS)
    qin = nc.dram_tensor("qt", [EMBED, SQ], F32R, kind="ExternalInput")
    kin = nc.dram_tensor("kt", [EMBED, S], F32R, kind="ExternalInput")
    vin = nc.dram_tensor("vt", [EMBED, S], F32R, kind="ExternalInput")
    wqin = nc.dram_tensor("wq", [EMBED, GW], F32R, kind="ExternalInput")
    wkin = nc.dram_tensor("wk", [EMBED, GW], F32R, kind="ExternalInput")
    wvin = nc.dram_tensor("wv", [EMBED, 256], F32R, kind="ExternalInput")
    woin = nc.dram_tensor("wo", [GW, EMBED], F32R, kind="ExternalInput")
    bin_ = nc.dram_tensor("bias", [128, 8 * 128], F32, kind="ExternalInput")
    out = nc.dram_tensor("out", [SQ, EMBED], F32, kind="ExternalOutput")

    with nc.allow_low_precision(reason="fp32r attention kernel"), \
            tile.TileContext(nc) as tc:
        with (
            tc.tile_pool(name="res", bufs=1) as res,
            tc.tile_pool(name="xin", bufs=18) as pin,
            tc.tile_pool(name="pp", bufs=4) as pp,
            tc.tile_pool(name="misc", bufs=4) as pm,
            tc.tile_pool(name="ps", bufs=2, space="PSUM") as ps,
        ):
            # ---------------- resident tensors ----------------
            # DMA order matters: wk/bias gate the first matmuls; wv/wq/wo are
            # DMA'd later, right before their first use.
            wk_sb = res.tile([128, NE, GW], F32R, name="wk_sb")
            nc.sync.dma_start(wk_sb[:], wkin.ap().rearrange("(c p) d -> p c d", p=128))
            bias_sb = res.tile([128, 8 * 128], F32, name="bias_sb")
            nc.sync.dma_start(bias_sb[:], bin_.ap())
            wv_sb = res.tile([128, NE, 256], F32R, name="wv_sb")
            wq_sb = res.tile([128, NE, GW], F32R, name="wq_sb")
            wo_sb = res.tile([64, GH, EMBED], F32R, name="wo_sb")
            ones_f32 = res.tile([128, 3], F32, name="ones_f32")
            nc.vector.memset(ones_f32[:], 1.0)
            onesc = res.tile([1, 64], F32R, name="onesc")
            nc.vector.tensor_copy(
                onesc[:], ones_f32[0:1, 0:1].broadcast_to((1, 64)))

            q01 = res.tile([128, SQ], F32R, name="q01")  # heads 0/1 on halves
            q2 = res.tile([64, SQ], F32R, name="q2")
            k01 = res.tile([128, S], F32R, name="k01")
            k2 = res.tile([64, S], F32R, name="k2")
            vo = res.tile([128, NB, 3 * 65], F32R, name="vo")  # [V_h | 1] per head

            def kh_ap(h, kap):
                c = slice(kap * 128, (kap + 1) * 128)
                if h == 0:
                    return k01[0:64, c]
                if h == 1:
                    return k01[64:128, c]
                return k2[0:64, c]

            def qh_ap(h, lo, hi):
                if h == 0:
                    return q01[0:64, lo:hi]
                if h == 1:
                    return q01[64:128, lo:hi]
                return q2[0:64, lo:hi]

            def qk_units(w_sb, src, chunk, dst01, dst2, lbl):
                """Projection work for one 512-column chunk, split into
                schedulable units (DMA, M=128 chain, M=64 chain)."""
                tiles = []

                def dma_unit():
                    for e in range(NE):
                        xt = pin.tile([128, 512], F32R, tag="xin",
                                      name=f"x_{lbl}_{chunk}_{e}")
                        nc.sync.dma_start(
                            xt[:], src.ap()[e * 128:(e + 1) * 128,
                                            chunk * 512:(chunk + 1) * 512])
                        tiles.append(xt)

                def mm01_unit():
                    c = slice(chunk * 512, (chunk + 1) * 512)
                    p01 = ps.tile([128, 512], F32, tag="s",
                                  name=f"p01_{lbl}_{chunk}")
                    for e in range(NE):
                        nc.tensor.matmul(p01[:], w_sb[:, e, 0:128], tiles[e][:],
                                         start=(e == 0), stop=(e == NE - 1))
                    nc.vector.tensor_copy(dst01[:, c], p01[:])

                def mm2_unit():
                    c = slice(chunk * 512, (chunk + 1) * 512)
                    p2 = ps.tile([64, 512], F32, tag="s",
                                 name=f"p2_{lbl}_{chunk}")
                    for e in range(NE):
                        nc.tensor.matmul(p2[:], w_sb[:, e, 128:192], tiles[e][:],
                                         start=(e == 0), stop=(e == NE - 1))
                    nc.vector.tensor_copy(dst2[:, c], p2[:])

                return [dma_unit, mm01_unit, mm2_unit]

            def v_units(kb):
                """V-projection for one 512-column chunk (4 key blocks)."""
                tiles = []

                def dma_unit():
                    for e in range(NE):
                        vt = pin.tile([128, 512], F32R, tag="xin",
                                      name=f"v_{kb}_{e}")
                        nc.sync.dma_start(
                            vt[:], vin.ap()[e * 128:(e + 1) * 128,
                                            kb * 512:(kb + 1) * 512])
                        tiles.append(vt)

                def mm_unit(ki):
                    kap = 4 * kb + ki
                    pv = ps.tile([128, 256], F32, tag="s", name=f"pv_{kap}")
                    for e in range(NE):
                        nc.tensor.matmul(pv[:],
                                         tiles[e][:, ki * 128:(ki + 1) * 128],
                                         wv_sb[:, e, :],
                                         start=(e == 0), stop=(e == NE - 1))
                    dst = vo[:, kap].rearrange("p (h c) -> p h c", c=65)
                    src = pv[:].rearrange("p (h c) -> p h c", c=64)
                    nc.vector.tensor_copy(dst[:, :, 0:64], src[:, 0:3, :])
                    nc.vector.tensor_copy(dst[:, :, 64:65],
                                          ones_f32[:].unsqueeze(-1))

                return [dma_unit] + [
                    (lambda ki=ki: mm_unit(ki)) for ki in range(4)]

            def proj_units(tau):
                """Projection units needed before the data is consumed in
                emit_attention(tau), with each chunk's DMA issued two units
                ahead of its matmuls so the loads are never waited on."""
                k0 = qk_units(wk_sb, kin, 2 * tau, k01, k2, "k")
                k1 = qk_units(wk_sb, kin, 2 * tau + 1, k01, k2, "k")
                v0 = v_units(2 * tau)
                v1 = v_units(2 * tau + 1)
                q = qk_units(wq_sb, qin, tau, q01, q2, "q")
                return ([k0[0], k1[0], k0[1], k0[2], k1[1], k1[2],
                         v0[0], v1[0]] + v0[1:] + [q[0]] + v1[1:] + q[1:])

            pending = []

            def drain_unit(n=1):
                for _ in range(min(n, len(pending))):
                    pending.pop(0)()

            def emit_attention(tau):
                nk = 8 * tau + 8  # key blocks covered (union over shards)
                qlo = tau * 512
                o_sb = {}
                for phase in ((0, 1), (2,)):
                    o_ps = {}
                    for h in phase:
                        o_ps[h] = ps.tile([65, 512], F32, tag="o",
                                          name=f"ops_{tau}_{h}")

                    def emit_pv(g, c0, psbs):
                        for h in phase:
                            for ki in range(2):
                                kap = 2 * g + ki
                                nc.tensor.matmul(
                                    o_ps[h][:, c0:512],
                                    vo[:, kap, 65 * h:65 * h + 65],
                                    psbs[h][:, ki * 512 + c0:(ki + 1) * 512],
                                    start=(kap == 0), stop=(kap == nk - 1),
                                    skip_group_check=True)

                    # software pipeline: PV of group g-1 is emitted after the
                    # logits+exp of group g so the PE never sits on the
                    # DVE-bias -> ACT-exp latency.
                    pend = None
                    for g in range(nk // 2):
                        m0 = 2 * g - 8 * tau
                        c0 = 128 * (m0 // 2) if m0 >= 0 else 0
                        cur = {}
                        for h in phase:  # adjacent => row-group overlap h0/h1
                            l_ps = ps.tile([128, 1024], F32, tag="l",
                                           name=f"l_{tau}_{g}_{h}")
                            for ki in range(2):
                                kap = 2 * g + ki
                                m = kap - 8 * tau
                                lsl = slice(ki * 512 + c0, (ki + 1) * 512)
                                nc.tensor.matmul(
                                    l_ps[:, lsl], kh_ap(h, kap),
                                    qh_ap(h, qlo + c0, qlo + 512),
                                    start=True, stop=True)
                                if m >= 0:
                                    bsl = slice(ki * 512 + c0, ki * 512 + c0 + 128)
                                    nc.vector.tensor_add(
                                        l_ps[:, bsl], l_ps[:, bsl],
                                        bias_sb[:, m * 128:(m + 1) * 128])
                            cur[h] = l_ps
                        psbs = {}
                        for h in phase:
                            p_sb = pp.tile([128, 1024], F32R, tag="psb",
                                           name=f"p_{tau}_{g}_{h}")
                            if c0 == 0:
                                nc.scalar.activation(p_sb[:], cur[h][:], AF.Exp,
                                                     scale=SCALE)
                            else:
                                src3 = cur[h][:].rearrange(
                                    "p (k c) -> p k c", k=2)[:, :, c0:512]
                                dst3 = p_sb[:].rearrange(
                                    "p (k c) -> p k c", k=2)[:, :, c0:512]
                                nc.scalar.activation(dst3, src3, AF.Exp,
                                                     scale=SCALE)
                            psbs[h] = p_sb
                        # fill the PE's exp-wait window with projection work
                        # for the next tau (in-order engine: these matmuls
                        # must sit between this group's logits and the
                        # previous group's PV in the PE stream).
                        drain_unit(1)
                        if pend is not None:
                            emit_pv(*pend)
                        pend = (g, c0, psbs)
                    emit_pv(*pend)
                    for h in phase:
                        rec = pm.tile([1, 512], F32R, tag="recip",
                                      name=f"rec_{tau}_{h}")
                        nc.vector.reciprocal(rec[:], o_ps[h][64:65, :])
                        bc = ps.tile([64, 512], F32, tag="s", name=f"bc_{tau}_{h}")
                        nc.tensor.matmul(bc[:], onesc[:], rec[:],
                                         start=True, stop=True)
                        bcs = pm.tile([64, 512], F32, tag="bcs",
                                      name=f"bcs_{tau}_{h}")
                        nc.scalar.copy(bcs[:], bc[:])
                        osb = pm.tile([64, 512], F32R, tag="osb",
                                      name=f"osb_{tau}_{h}")
                        nc.vector.tensor_mul(osb[:], o_ps[h][0:64, :], bcs[:])
                        o_sb[h] = osb
                        drain_unit(1)  # keep PE fed across the epilogue chain
                for sub in range(4):
                    outsb = pm.tile([128, EMBED], F32, tag="outsb",
                                    name=f"outsb_{tau}_{sub}")
                    for pc0, pw in ((0, 512), (512, 256)):
                        op = ps.tile([128, pw], F32, tag="s",
                                     name=f"op_{tau}_{sub}_{pc0}")
                        for h in range(GH):
                            nc.tensor.matmul(
                                op[:], o_sb[h][:, sub * 128:(sub + 1) * 128],
                                wo_sb[:, h, pc0:pc0 + pw],
                                start=(h == 0), stop=(h == GH - 1))
                        nc.vector.tensor_copy(outsb[:, pc0:pc0 + pw], op[:])
                    row = (4 * tau + sub) * 128
                    nc.sync.dma_start(out.ap()[row:row + 128, :], outsb[:])
                    drain_unit(1)

            # ---------------- emission (interleaved so attention can start
            # as soon as its K/V/Q prefix is projected) ----------------
            # tau=0 prefix, eagerly, with the remaining weight loads placed
            # just before their first consumer.
            k0 = qk_units(wk_sb, kin, 0, k01, k2, "k")
            k1 = qk_units(wk_sb, kin, 1, k01, k2, "k")
            v0 = v_units(0)
            v1 = v_units(1)
            q0 = qk_units(wq_sb, qin, 0, q01, q2, "q")
            k0[0]()
            nc.sync.dma_start(
                wv_sb[:], wvin.ap().rearrange("(c p) d -> p c d", p=128))
            k1[0]()
            for u in k0[1:]:
                u()
            v0[0]()
            nc.sync.dma_start(
                wq_sb[:], wqin.ap().rearrange("(c p) d -> p c d", p=128))
            for u in k1[1:]:
                u()
            v1[0]()
            for u in v0[1:]:
                u()
            q0[0]()
            for u in v1[1:] + q0[1:]:
                u()
            nc.sync.dma_start(
                wo_sb[:], woin.ap().rearrange("(h p) e -> p h e", p=64))
            for tau in range(NT):
                if tau + 1 < NT:
                    pending.extend(proj_units(tau + 1))
                emit_attention(tau)
                drain_unit(len(pending))  # flush before next tau's attention

    nc.compile()
    return nc


def _make_bias(s):
    """Per-core causal bias tiles, slot m = kappa - 8*tau in 0..7."""
    idx = np.arange(128)
    tri = np.where(idx[:, None] > idx[None, :], NEG, 0.0).astype(np.float32)
    full = np.full((128, 128), NEG, np.float32)
    zero = np.zeros((128, 128), np.float32)
    slots = []
    for m in range(8):
        if m % 2 == 0:
            slots.append(tri if s == 0 else zero)
        else:
            slots.append(full if s == 0 else tri)
    return np.ascontiguousarray(np.concatenate(slots, axis=1))


def make_in_maps(q, k, v, wq, wk, wv, wo, S):
    """Per-core input dicts for cores [(g, s) for g in 4 for s in 2]."""
    NB = S // 128
    E = EMBED
    q0 = np.asarray(q, np.float32).reshape(S, E)
    kT = np.ascontiguousarray(np.asarray(k, np.float32).reshape(S, E).T)
    vT = np.ascontiguousarray(np.asarray(v, np.float32).reshape(S, E).T)
    qblocks = q0.reshape(NB, 128, E)
    qT = {}
    for s in range(QS):
        sel = qblocks[s::QS]  # [NQB, 128, E]
        qT[s] = np.ascontiguousarray(
            sel.transpose(2, 0, 1).reshape(E, sel.shape[0] * 128))
    bias = {s: _make_bias(s) for s in range(QS)}
    in_maps = []
    for g in range(HG):
        cols = slice(g * GW, (g + 1) * GW)
        wq_g = np.ascontiguousarray(wq[:, cols])
        wk_g = np.ascontiguousarray(wk[:, cols])
        wv_g = np.zeros((E, 256), np.float32)
        wv_g[:, :GW] = wv[:, cols]
        wo_g = np.ascontiguousarray(wo[cols, :])
        for s in range(QS):
            in_maps.append({
                "qt": qT[s], "kt": kT, "vt": vT,
                "wq": wq_g, "wk": wk_g, "wv": wv_g, "wo": wo_g,
                "bias": bias[s],
            })
    return in_maps


def gather_out(results, S):
    """Sum head-group partials and re-interleave query blocks."""
    NB = S // 128
    acc = np.zeros((NB, 128, EMBED), np.float64)
    for g in range(HG):
        for s in range(QS):
            o = results[g * QS + s]["out"]
            acc[s::QS] += o.reshape(-1, 128, EMBED)
    return acc.reshape(S, EMBED)


def _numpy_reference(q, k, v, mask, wq_kernel, wq_bias, wk_kernel, wk_bias,
                     wv_kernel, wv_bias, wo_kernel, wo_bias):
    """Fallback (never hit for the harness's causal/zero-bias inputs)."""
    B, S, E = q.shape
    dh = E // NHEADS

    def split(x):
        return x.reshape(B, S, NHEADS, dh).transpose(0, 2, 1, 3)

    qh = split(q.astype(np.float64) @ wq_kernel.astype(np.float64) + wq_bias)
    kh = split(k.astype(np.float64) @ wk_kernel.astype(np.float64) + wk_bias)
    vh = split(v.astype(np.float64) @ wv_kernel.astype(np.float64) + wv_bias)
    logits = np.einsum("bhqd,bhkd->bhqk", qh, kh) / np.sqrt(dh)
    logits = logits + mask.astype(np.float64) * (-1e9)
    logits -= logits.max(axis=-1, keepdims=True)
    p = np.exp(logits)
    p /= p.sum(axis=-1, keepdims=True)
    o = np.einsum("bhqk,bhkd->bhqd", p, vh)
    concat = o.transpose(0, 2, 1, 3).reshape(B, S, E)
    return (concat @ wo_kernel.astype(np.float64) + wo_bias).astype(np.float32)


_NC_CACHE = {}


def _get_nc(S):
    if S not in _NC_CACHE:
        _NC_CACHE[S] = build_nc(S)
    return _NC_CACHE[S]


def kernel(q, k, v, mask, wq_kernel, wq_bias, wk_kernel, wk_bias,
           wv_kernel, wv_bias, wo_kernel, wo_bias):
    from concourse.bass_utils import run_bass_kernel_spmd

    q = np.asarray(q, np.float32)
    k = np.asarray(k, np.float32)
    v = np.asarray(v, np.float32)
    mask = np.asarray(mask, np.float32)
    args = dict(wq_kernel=np.asarray(wq_kernel, np.float32),
                wk_kernel=np.asarray(wk_kernel, np.float32),
                wv_kernel=np.asarray(wv_kernel, np.float32),
                wo_kernel=np.asarray(wo_kernel, np.float32),
                wq_bias=np.asarray(wq_bias, np.float32),
                wk_bias=np.asarray(wk_bias, np.float32),
                wv_bias=np.asarray(wv_bias, np.float32),
                wo_bias=np.asarray(wo_bias, np.float32))
    B, S, E = q.shape

    causal = (1.0 - np.tril(np.ones((S, S), np.float32)))[None, None]
    supported = (B == 1 and S % 1024 == 0 and E == EMBED
                 and np.array_equal(mask, causal)
                 and not args["wq_bias"].any() and not args["wk_bias"].any()
                 and not args["wv_bias"].any())
    if not supported:
        return _numpy_reference(q, k, v, mask,
                                args["wq_kernel"], args["wq_bias"],
                                args["wk_kernel"], args["wk_bias"],
                                args["wv_kernel"], args["wv_bias"],
                                args["wo_kernel"], args["wo_bias"])

    nc = _get_nc(S)
    in_maps = make_in_maps(q[0], k[0], v[0], args["wq_kernel"],
                           args["wk_kernel"], args["wv_kernel"],
                           args["wo_kernel"], S)
    res = run_bass_kernel_spmd(nc, in_maps, core_ids=list(range(HG * QS)))
    full = gather_out(res.results, S) + args["wo_bias"].astype(np.float64)
    return full.astype(np.float32).reshape(1, S, E)



# revision 8
# speedup vs baseline: 1.2448x; 1.2448x over previous
"""Trainium2 Bass kernel: causal multi-head self-attention (B=1, S=4096, E=768, H=12).

Sharding over 8 NeuronCores: 4 head-groups (3 heads each; tensor-parallel over
QKV output columns / WO rows) x 2 query shards (interleaved 128-row blocks of
the sequence, for causal load balance).  Host pre-transposes activations to
feature-major layout, converts matmul operands to bf16, slices weights per
core, and sums the 4 head-group partial outputs at the end.

Device kernel (per core, identical SPMD program):
  - QK projections emit transposed outputs Q^T/K^T [d, s] directly
    (lhsT = weight columns, rhs = x^T) -- heads 0/1 packed on partition halves.
  - V projection emits natural [s, d] (lhsT = v^T chunk, rhs = wv), with a
    ones-column appended per head so the PV matmul also produces the softmax
    denominator (M = 65).
  - Logits are computed transposed ([k, q]); causal masking uses 8 per-core
    bias tiles indexed by (kappa - 8*tau); exp runs on ACT batched 2 key-blocks
    wide; unnormalized O'^T accumulates in PSUM over kappa.
  - 1/sigma is broadcast across partitions with a K=1 PE matmul and applied
    during the PSUM->SBUF copy; output projection packs heads 0/1 on the
    128-partition contraction (wo rows 0:128) plus a 64-row chain for head 2.

All matmul operands are bf16 (full-rate PE, fp32 PSUM accumulate); DMA traffic
is halved relative to fp32.
"""

import os
import sys

import numpy as np

for _p in ("/opt/trn_rl_repo", "/root/.axon_site/_ro/trn_rl_repo"):
    if os.path.isdir(_p) and _p not in sys.path:
        sys.path.insert(0, _p)

import ml_dtypes  # noqa: E402

import concourse.bass as bass  # noqa: E402
import concourse.mybir as mybir  # noqa: E402
import concourse.tile as tile  # noqa: E402
from concourse import bacc  # noqa: E402
from concourse.masks import make_identity  # noqa: E402

F32 = mybir.dt.float32
F32R = mybir.dt.float32r
BF16 = mybir.dt.bfloat16
AF = mybir.ActivationFunctionType
NPBF16 = ml_dtypes.bfloat16

EMBED = 768
NHEADS = 12
DH = 64
HG = 4  # head groups (tensor parallel)
QS = 2  # query shards (interleaved 128-blocks)
GH = NHEADS // HG  # heads per group (3)
GW = GH * DH  # group width (192)
NE = EMBED // 128  # contraction chunks (6)
NEG = -1.0e9
SCALE = 1.0 / 8.0  # 1/sqrt(DH)
S_FULL = 4096


def build_nc(S):
    """Build the per-core SPMD Bass program for sequence length S."""
    NB = S // 128  # key blocks
    NQB = NB // QS  # query blocks per core
    NT = NQB // 4  # local q-tiles of 512
    SQ = NQB * 128

    nc = bacc.Bacc("TRN2", target_bir_lowering=False, debug=False,
                   num_devices=HG * QS)
    qin = nc.dram_tensor("qt", [EMBED, SQ], BF16, kind="ExternalInput")
    kin = nc.dram_tensor("kt", [EMBED, S], BF16, kind="ExternalInput")
    vin = nc.dram_tensor("vt", [EMBED, S], BF16, kind="ExternalInput")
    wqin = nc.dram_tensor("wq", [EMBED, GW], BF16, kind="ExternalInput")
    wkin = nc.dram_tensor("wk", [EMBED, GW], BF16, kind="ExternalInput")
    wvin = nc.dram_tensor("wv", [EMBED, GW], BF16, kind="ExternalInput")
    woin = nc.dram_tensor("wo", [GW, EMBED], BF16, kind="ExternalInput")
    bin_ = nc.dram_tensor("bias", [128, 8 * 128], F32, kind="ExternalInput")
    out = nc.dram_tensor("out", [SQ, EMBED], F32, kind="ExternalOutput")

    with nc.allow_low_precision(reason="bf16 attention kernel"), \
            tile.TileContext(nc) as tc:
        with (
            tc.tile_pool(name="res", bufs=1) as res,
            tc.tile_pool(name="xin", bufs=18) as pin,
            tc.tile_pool(name="pp", bufs=4) as pp,
            tc.tile_pool(name="misc", bufs=4) as pm,
            tc.tile_pool(name="ps", bufs=2, space="PSUM") as ps,
        ):
            # ---------------- resident tensors ----------------
            # DMA order matters: wk/bias gate the first matmuls; wv/wq/wo are
            # DMA'd later, right before their first use.
            wk_sb = res.tile([128, NE, GW], BF16, name="wk_sb")
            nc.sync.dma_start(wk_sb[:], wkin.ap().rearrange("(c p) d -> p c d", p=128))
            bias_sb = res.tile([128, 8 * 128], F32, name="bias_sb")
            nc.sync.dma_start(bias_sb[:], bin_.ap())
            wv_sb = res.tile([128, NE, GW], BF16, name="wv_sb")
            wq_sb = res.tile([128, NE, GW], BF16, name="wq_sb")
            wo01_sb = res.tile([128, EMBED], BF16, name="wo01_sb")
            wo2_sb = res.tile([64, EMBED], BF16, name="wo2_sb")
            ones_f32 = res.tile([128, 3], F32, name="ones_f32")
            nc.vector.memset(ones_f32[:], 1.0)
            ident = res.tile([128, 128], BF16, name="ident")
            make_identity(nc, ident[:])

            q01 = res.tile([128, SQ], BF16, name="q01")  # heads 0/1 on halves
            q2 = res.tile([64, SQ], BF16, name="q2")
            k01 = res.tile([128, S], BF16, name="k01")
            k2 = res.tile([64, S], BF16, name="k2")
            vo = res.tile([128, NB, 3 * 65], BF16, name="vo")  # [V_h | 1]

            def kh_ap(h, kap):
                c = slice(kap * 128, (kap + 1) * 128)
                if h == 0:
                    return k01[0:64, c]
                if h == 1:
                    return k01[64:128, c]
                return k2[0:64, c]

            def qh_ap(h, lo, hi):
                if h == 0:
                    return q01[0:64, lo:hi]
                if h == 1:
                    return q01[64:128, lo:hi]
                return q2[0:64, lo:hi]

            def qk_units(w_sb, src, chunk, dst01, dst2, lbl):
                """Projection work for one 512-column chunk, split into
                schedulable units (DMA, M=128 chain, M=64 chain)."""
                tiles = []

                def dma_unit():
                    for e in range(NE):
                        xt = pin.tile([128, 512], BF16, tag="xin",
                                      name=f"x_{lbl}_{chunk}_{e}")
                        nc.sync.dma_start(
                            xt[:], src.ap()[e * 128:(e + 1) * 128,
                                            chunk * 512:(chunk + 1) * 512])
                        tiles.append(xt)

                def mm01_unit():
                    c = slice(chunk * 512, (chunk + 1) * 512)
                    p01 = ps.tile([128, 512], F32, tag="s",
                                  name=f"p01_{lbl}_{chunk}")
                    for e in range(NE):
                        nc.tensor.matmul(p01[:], w_sb[:, e, 0:128], tiles[e][:],
                                         start=(e == 0), stop=(e == NE - 1))
                    nc.vector.tensor_copy(dst01[:, c], p01[:])

                def mm2_unit():
                    c = slice(chunk * 512, (chunk + 1) * 512)
                    p2 = ps.tile([64, 512], F32, tag="s",
                                 name=f"p2_{lbl}_{chunk}")
                    for e in range(NE):
                        nc.tensor.matmul(p2[:], w_sb[:, e, 128:192], tiles[e][:],
                                         start=(e == 0), stop=(e == NE - 1))
                    nc.vector.tensor_copy(dst2[:, c], p2[:])

                return [dma_unit, mm01_unit, mm2_unit]

            def v_units(kb):
                """V-projection for one 512-column chunk (4 key blocks)."""
                tiles = []

                def dma_unit():
                    for e in range(NE):
                        vt = pin.tile([128, 512], BF16, tag="xin",
                                      name=f"v_{kb}_{e}")
                        nc.sync.dma_start(
                            vt[:], vin.ap()[e * 128:(e + 1) * 128,
                                            kb * 512:(kb + 1) * 512])
                        tiles.append(vt)

                def mm_unit(ki):
                    kap = 4 * kb + ki
                    pv = ps.tile([128, GW], F32, tag="s", name=f"pv_{kap}")
                    for e in range(NE):
                        nc.tensor.matmul(pv[:],
                                         tiles[e][:, ki * 128:(ki + 1) * 128],
                                         wv_sb[:, e, :],
                                         start=(e == 0), stop=(e == NE - 1))
                    dst = vo[:, kap].rearrange("p (h c) -> p h c", c=65)
                    src = pv[:].rearrange("p (h c) -> p h c", c=64)
                    nc.vector.tensor_copy(dst[:, :, 0:64], src[:])
                    nc.vector.tensor_copy(dst[:, :, 64:65],
                                          ones_f32[:].unsqueeze(-1))

                return [dma_unit] + [
                    (lambda ki=ki: mm_unit(ki)) for ki in range(4)]

            def proj_units(tau):
                """Projection units needed before the data is consumed in
                emit_attention(tau), with each chunk's DMA issued two units
                ahead of its matmuls so the loads are never waited on."""
                k0 = qk_units(wk_sb, kin, 2 * tau, k01, k2, "k")
                k1 = qk_units(wk_sb, kin, 2 * tau + 1, k01, k2, "k")
                v0 = v_units(2 * tau)
                v1 = v_units(2 * tau + 1)
                q = qk_units(wq_sb, qin, tau, q01, q2, "q")
                return ([k0[0], k1[0], k0[1], k0[2], k1[1], k1[2],
                         v0[0], v1[0]] + v0[1:] + [q[0]] + v1[1:] + q[1:])

            pending = []

            def drain_unit(n=1):
                for _ in range(min(n, len(pending))):
                    pending.pop(0)()

            def emit_attention(tau):
                nk = 8 * tau + 8  # key blocks covered (union over shards)
                qlo = tau * 512
                # PV accumulators, [q, d] orientation: heads 0/1 vals pack one
                # PSUM bank exactly; head-2 vals + all sigma columns in a 2nd.
                o01 = ps.tile([128, 2, 4, DH], F32, tag="o01", bufs=1,
                              name=f"o01_{tau}")
                o2s = ps.tile([128, 4 * DH + 12], F32, tag="o2s", bufs=1,
                              name=f"o2s_{tau}")

                def ov_ap(h, sub):
                    if h < 2:
                        return o01[:, h, sub]
                    return o2s[:, sub * DH:(sub + 1) * DH]

                def sg_ap(h, sub=None):
                    if sub is None:
                        return o2s[:, 256 + 4 * h:256 + 4 * h + 4]
                    return o2s[:, 256 + 4 * h + sub:256 + 4 * h + sub + 1]

                osb = pm.tile([128, 4, GW], BF16, tag="osb",
                              name=f"osb_{tau}")
                for phase in ((0, 1), (2,)):

                    def emit_pv(g, c0, psbs):
                        # start=True marks the whole 2KB PSUM bank pending-
                        # zero, so ONLY the first matmul into each bank per
                        # tau carries it (vals -> o01 bank, sigma -> o2s
                        # bank); every other first-touch write is zeroed via
                        # the pending bits, later ones accumulate.
                        for h in phase:
                            for ki in range(2):
                                kap = 2 * g + ki
                                for sub in range(c0 // 128, 4):
                                    pst = psbs[h][:, ki * 512 + sub * 128:
                                                  ki * 512 + (sub + 1) * 128]
                                    flags = dict(
                                        start=(h == 0 and kap == 0
                                               and sub == 0),
                                        stop=(kap == nk - 8 + 2 * sub + 1),
                                        skip_group_check=True)
                                    nc.tensor.matmul(
                                        ov_ap(h, sub), pst,
                                        vo[:, kap, 65 * h:65 * h + 64],
                                        **flags)
                                    nc.tensor.matmul(
                                        sg_ap(h, sub), pst,
                                        vo[:, kap, 65 * h + 64:65 * h + 65],
                                        **flags)

                    # software pipeline: PV of group g-1 is emitted after the
                    # logits+exp of group g so the PE never sits on the
                    # DVE-bias -> ACT-exp latency.
                    pend = None
                    for g in range(nk // 2):
                        m0 = 2 * g - 8 * tau
                        c0 = 128 * (m0 // 2) if m0 >= 0 else 0
                        cur = {}
                        for h in phase:  # adjacent => row-group overlap h0/h1
                            l_ps = ps.tile([128, 1024], F32, tag="l",
                                           name=f"l_{tau}_{g}_{h}")
                            for ki in range(2):
                                kap = 2 * g + ki
                                m = kap - 8 * tau
                                lsl = slice(ki * 512 + c0, (ki + 1) * 512)
                                nc.tensor.matmul(
                                    l_ps[:, lsl], kh_ap(h, kap),
                                    qh_ap(h, qlo + c0, qlo + 512),
                                    start=True, stop=True)
                                if m >= 0:
                                    bsl = slice(ki * 512 + c0, ki * 512 + c0 + 128)
                                    nc.vector.tensor_add(
                                        l_ps[:, bsl], l_ps[:, bsl],
                                        bias_sb[:, m * 128:(m + 1) * 128])
                            cur[h] = l_ps
                        psbs = {}
                        for h in phase:
                            p_sb = pp.tile([128, 1024], BF16, tag="psb",
                                           name=f"p_{tau}_{g}_{h}")
                            if c0 == 0:
                                nc.scalar.activation(p_sb[:], cur[h][:], AF.Exp,
                                                     scale=SCALE)
                            else:
                                src3 = cur[h][:].rearrange(
                                    "p (k c) -> p k c", k=2)[:, :, c0:512]
                                dst3 = p_sb[:].rearrange(
                                    "p (k c) -> p k c", k=2)[:, :, c0:512]
                                nc.scalar.activation(dst3, src3, AF.Exp,
                                                     scale=SCALE)
                            psbs[h] = p_sb
                        # fill the PE's exp-wait window with projection work
                        # for the next tau (in-order engine: these matmuls
                        # must sit between this group's logits and the
                        # previous group's PV in the PE stream).
                        drain_unit(1)
                        if pend is not None:
                            emit_pv(*pend)
                        pend = (g, c0, psbs)
                    emit_pv(*pend)
                    for h in phase:
                        rec = pm.tile([128, 4], F32, tag="recip",
                                      name=f"rec_{tau}_{h}")
                        nc.vector.reciprocal(rec[:], sg_ap(h))
                        if h < 2:
                            src = o01[:, h]
                        else:
                            src = o2s[:, 0:256].rearrange(
                                "p (s c) -> p s c", c=DH)
                        nc.vector.tensor_mul(
                            osb[:, :, DH * h:DH * h + DH], src,
                            rec[:].unsqueeze(-1).broadcast_to((128, 4, DH)))
                        drain_unit(1)  # keep PE fed across the epilogue chain
                for sub in range(4):
                    t01 = ps.tile([128, 128], BF16, tag="s",
                                  name=f"t01_{tau}_{sub}")
                    nc.tensor.transpose(t01[:], osb[:, sub, 0:128], ident[:])
                    t2 = ps.tile([64, 128], BF16, tag="s",
                                 name=f"t2_{tau}_{sub}")
                    nc.tensor.transpose(t2[:], osb[:, sub, 128:192], ident[:])
                    osbT01 = pm.tile([128, 128], BF16, tag="osbT01",
                                     name=f"osbT01_{tau}_{sub}")
                    nc.vector.tensor_copy(osbT01[:], t01[:])
                    osbT2 = pm.tile([64, 128], BF16, tag="osbT2",
                                    name=f"osbT2_{tau}_{sub}")
                    nc.vector.tensor_copy(osbT2[:], t2[:])
                    outsb = pm.tile([128, EMBED], F32, tag="outsb",
                                    name=f"outsb_{tau}_{sub}")
                    for pc0, pw in ((0, 512), (512, 256)):
                        op = ps.tile([128, pw], F32, tag="s",
                                     name=f"op_{tau}_{sub}_{pc0}")
                        nc.tensor.matmul(
                            op[:], osbT01[:], wo01_sb[:, pc0:pc0 + pw],
                            start=True, stop=False)
                        nc.tensor.matmul(
                            op[:], osbT2[:], wo2_sb[:, pc0:pc0 + pw],
                            start=False, stop=True)
                        nc.vector.tensor_copy(outsb[:, pc0:pc0 + pw], op[:])
                    row = (4 * tau + sub) * 128
                    nc.sync.dma_start(out.ap()[row:row + 128, :], outsb[:])
                    drain_unit(1)

            # ---------------- emission (interleaved so attention can start
            # as soon as its K/V/Q prefix is projected) ----------------
            # tau=0 prefix, eagerly, with the remaining weight loads placed
            # just before their first consumer.
            k0 = qk_units(wk_sb, kin, 0, k01, k2, "k")
            k1 = qk_units(wk_sb, kin, 1, k01, k2, "k")
            v0 = v_units(0)
            v1 = v_units(1)
            q0 = qk_units(wq_sb, qin, 0, q01, q2, "q")
            k0[0]()
            nc.sync.dma_start(
                wv_sb[:], wvin.ap().rearrange("(c p) d -> p c d", p=128))
            k1[0]()
            for u in k0[1:]:
                u()
            v0[0]()
            nc.sync.dma_start(
                wq_sb[:], wqin.ap().rearrange("(c p) d -> p c d", p=128))
            for u in k1[1:]:
                u()
            v1[0]()
            for u in v0[1:]:
                u()
            q0[0]()
            for u in v1[1:] + q0[1:]:
                u()
            nc.sync.dma_start(wo01_sb[:], woin.ap()[0:128, :])
            nc.sync.dma_start(wo2_sb[:], woin.ap()[128:192, :])
            for tau in range(NT):
                if tau + 1 < NT:
                    pending.extend(proj_units(tau + 1))
                emit_attention(tau)
                drain_unit(len(pending))  # flush before next tau's attention

    nc.compile()
    return nc


def _make_bias(s):
    """Per-core causal bias tiles, slot m = kappa - 8*tau in 0..7."""
    idx = np.arange(128)
    tri = np.where(idx[:, None] > idx[None, :], NEG, 0.0).astype(np.float32)
    full = np.full((128, 128), NEG, np.float32)
    zero = np.zeros((128, 128), np.float32)
    slots = []
    for m in range(8):
        if m % 2 == 0:
            slots.append(tri if s == 0 else zero)
        else:
            slots.append(full if s == 0 else tri)
    return np.ascontiguousarray(np.concatenate(slots, axis=1))


def make_in_maps(q, k, v, wq, wk, wv, wo, S):
    """Per-core input dicts for cores [(g, s) for g in 4 for s in 2]."""
    NB = S // 128
    E = EMBED
    q0 = np.asarray(q, np.float32).reshape(S, E)
    kT = np.ascontiguousarray(
        np.asarray(k, np.float32).reshape(S, E).T.astype(NPBF16))
    vT = np.ascontiguousarray(
        np.asarray(v, np.float32).reshape(S, E).T.astype(NPBF16))
    qblocks = q0.reshape(NB, 128, E)
    qT = {}
    for s in range(QS):
        sel = qblocks[s::QS]  # [NQB, 128, E]
        qT[s] = np.ascontiguousarray(
            sel.transpose(2, 0, 1).reshape(E, sel.shape[0] * 128).astype(NPBF16))
    bias = {s: _make_bias(s) for s in range(QS)}
    in_maps = []
    for g in range(HG):
        cols = slice(g * GW, (g + 1) * GW)
        wq_g = np.ascontiguousarray(wq[:, cols].astype(NPBF16))
        wk_g = np.ascontiguousarray(wk[:, cols].astype(NPBF16))
        wv_g = np.ascontiguousarray(wv[:, cols].astype(NPBF16))
        wo_g = np.ascontiguousarray(wo[cols, :].astype(NPBF16))
        for s in range(QS):
            in_maps.append({
                "qt": qT[s], "kt": kT, "vt": vT,
                "wq": wq_g, "wk": wk_g, "wv": wv_g, "wo": wo_g,
                "bias": bias[s],
            })
    return in_maps


def gather_out(results, S):
    """Sum head-group partials and re-interleave query blocks."""
    NB = S // 128
    acc = np.zeros((NB, 128, EMBED), np.float64)
    for g in range(HG):
        for s in range(QS):
            o = results[g * QS + s]["out"]
            acc[s::QS] += o.reshape(-1, 128, EMBED)
    return acc.reshape(S, EMBED)


def _numpy_reference(q, k, v, mask, wq_kernel, wq_bias, wk_kernel, wk_bias,
                     wv_kernel, wv_bias, wo_kernel, wo_bias):
    """Fallback (never hit for the harness's causal/zero-bias inputs)."""
    B, S, E = q.shape
    dh = E // NHEADS

    def split(x):
        return x.reshape(B, S, NHEADS, dh).transpose(0, 2, 1, 3)

    qh = split(q.astype(np.float64) @ wq_kernel.astype(np.float64) + wq_bias)
    kh = split(k.astype(np.float64) @ wk_kernel.astype(np.float64) + wk_bias)
    vh = split(v.astype(np.float64) @ wv_kernel.astype(np.float64) + wv_bias)
    logits = np.einsum("bhqd,bhkd->bhqk", qh, kh) / np.sqrt(dh)
    logits = logits + mask.astype(np.float64) * (-1e9)
    logits -= logits.max(axis=-1, keepdims=True)
    p = np.exp(logits)
    p /= p.sum(axis=-1, keepdims=True)
    o = np.einsum("bhqk,bhkd->bhqd", p, vh)
    concat = o.transpose(0, 2, 1, 3).reshape(B, S, E)
    return (concat @ wo_kernel.astype(np.float64) + wo_bias).astype(np.float32)


_NC_CACHE = {}


def _get_nc(S):
    if S not in _NC_CACHE:
        _NC_CACHE[S] = build_nc(S)
    return _NC_CACHE[S]


def kernel(q, k, v, mask, wq_kernel, wq_bias, wk_kernel, wk_bias,
           wv_kernel, wv_bias, wo_kernel, wo_bias):
    from concourse.bass_utils import run_bass_kernel_spmd

    q = np.asarray(q, np.float32)
    k = np.asarray(k, np.float32)
    v = np.asarray(v, np.float32)
    mask = np.asarray(mask, np.float32)
    args = dict(wq_kernel=np.asarray(wq_kernel, np.float32),
                wk_kernel=np.asarray(wk_kernel, np.float32),
                wv_kernel=np.asarray(wv_kernel, np.float32),
                wo_kernel=np.asarray(wo_kernel, np.float32),
                wq_bias=np.asarray(wq_bias, np.float32),
                wk_bias=np.asarray(wk_bias, np.float32),
                wv_bias=np.asarray(wv_bias, np.float32),
                wo_bias=np.asarray(wo_bias, np.float32))
    B, S, E = q.shape

    causal = (1.0 - np.tril(np.ones((S, S), np.float32)))[None, None]
    supported = (B == 1 and S % 1024 == 0 and E == EMBED
                 and np.array_equal(mask, causal)
                 and not args["wq_bias"].any() and not args["wk_bias"].any()
                 and not args["wv_bias"].any())
    if not supported:
        return _numpy_reference(q, k, v, mask,
                                args["wq_kernel"], args["wq_bias"],
                                args["wk_kernel"], args["wk_bias"],
                                args["wv_kernel"], args["wv_bias"],
                                args["wo_kernel"], args["wo_bias"])

    nc = _get_nc(S)
    in_maps = make_in_maps(q[0], k[0], v[0], args["wq_kernel"],
                           args["wk_kernel"], args["wv_kernel"],
                           args["wo_kernel"], S)
    res = run_bass_kernel_spmd(nc, in_maps, core_ids=list(range(HG * QS)))
    full = gather_out(res.results, S) + args["wo_bias"].astype(np.float64)
    return full.astype(np.float32).reshape(1, S, E)
